# revision 6
# baseline (speedup 1.0000x reference)
"""GCNConv-style GNN layer on 8 Trainium2 NeuronCores (Bass/Tile).

Reference computation (B=8, N=4096, C=128, E=131072):
    adj  = symmetric 0/1 adjacency from edge_index, zero diagonal
    h    = x @ W0 + b0
    agg  = adj @ h            (per batch)
    out  = (cat[x, agg] @ W1 + b1) @ W2 + b2
    out  = gelu(out) @ Wo + bo
    ret  = x + out

Algebraic refactor used here (all linear maps before the single GELU
compose; fold them on the host at O(C^2) cost):
    W12  = W1 @ W2                  [2C, C]
    Wx   = W12[:C]                  x-path weight
    Wa   = W0 @ W12[C:]             agg-path weight applied to s = adj @ x
    b0a  = b0 @ W12[C:]
    b12  = b1 @ W2 + b2
    pre  = x @ Wx + (adj @ x) @ Wa + deg ⊗ b0a + b12
    ret  = x + gelu(pre) @ Wo + bo
where deg = adj.sum(1) (the b0 bias aggregates to deg[i]*b0a).

Device work per core (node partition, NS=512 rows each, SPMD, no
collectives): the big matmul s = adj @ x_r with x_r in [node, (b,c)]
layout (K=4096 contraction, N_free=1024), a 128x128 PE transpose of s,
then the tiny fused MLP, all in bf16 with fp32 PSUM accumulation.
"""

import numpy as np
import ml_dtypes

import bass_rust
import concourse.bass as bass
import concourse.mybir as mybir
import concourse.tile as tile
from concourse.bass_utils import run_bass_kernel_spmd
from concourse.masks import make_identity

B, N, C, E = 8, 4096, 128, 131072
NCORES = 8
NS = N // NCORES          # 512 output rows per core
IC = NS // 128            # 4 i-chunks of 128 rows
KC = N // 128             # 32 k-chunks over the contraction dim
COLS = B * C              # 1024 columns of x_r  (b-major, c-minor)
RCOLS = B * NS            # 4096 columns of transposed row-space tiles

F32 = mybir.dt.float32
BF16 = mybir.dt.bfloat16
BF16_NP = ml_dtypes.bfloat16


def _split_multiwaits(nc, max_waits=1):
    """Walrus (CoreV3) refuses instructions with more than one sync wait.
    Tile's tail drain can carry several; hoist the extras onto preceding
    single-wait EventSemaphore instructions on the same engine."""
    for blk in nc.m.functions[0].blocks:
        new_list = []
        for ins in blk.instructions:
            si = ins.sync_info
            if si is not None and si.on_wait and len(si.on_wait) > max_waits:
                waits = list(si.on_wait)
                extra, keep = waits[:-max_waits], waits[-max_waits:]
                for i, w in enumerate(extra):
                    ev = mybir.InstEventSemaphore(
                        name=f"{ins.name}_wsplit{i}",
                        engine=ins.engine,
                        ins=[],
                        outs=[],
                        sync_info=bass_rust.SyncInfo(on_wait=[w], on_update=[]),
                    )
                    new_list.append(ev)
                si.on_wait = keep
            new_list.append(ins)
        blk.instructions[:] = new_list


def build_bass():
    nc = bass.Bass()

    xr_d = nc.dram_tensor("xr", [N, COLS], BF16, kind="ExternalInput")
    adjT_d = nc.dram_tensor("adjT", [N, NS], BF16, kind="ExternalInput")
    xt_bf_d = nc.dram_tensor("xt_bf", [C, RCOLS], BF16, kind="ExternalInput")
    xtbo_d = nc.dram_tensor("xtbo", [C, RCOLS], F32, kind="ExternalInput")
    degb0a_d = nc.dram_tensor("degb0a", [C, NS], F32, kind="ExternalInput")
    wx_d = nc.dram_tensor("wx", [C, C], BF16, kind="ExternalInput")
    wa_d = nc.dram_tensor("wa", [C, C], BF16, kind="ExternalInput")
    wo_d = nc.dram_tensor("wo", [C, C], BF16, kind="ExternalInput")
    b12_d = nc.dram_tensor("b12", [C, 1], F32, kind="ExternalInput")
    out_d = nc.dram_tensor("out", [C, RCOLS], F32, kind="ExternalOutput")

    with tile.TileContext(nc) as tc:
        with (
            tc.tile_pool(name="const", bufs=1) as const,
            tc.tile_pool(name="big", bufs=1) as big,
            tc.tile_pool(name="ps_s", bufs=3, space="PSUM") as ps_s,
            tc.tile_pool(name="ps_t", bufs=2, space="PSUM") as ps_t,
            tc.tile_pool(name="ps_pre", bufs=2, space="PSUM") as ps_pre,
            tc.tile_pool(name="ps_out", bufs=1, space="PSUM") as ps_out,
        ):
            # ---- resident inputs -------------------------------------
            xr_sb = big.tile([128, KC, COLS], BF16)
            adjT_sb = big.tile([128, KC, NS], BF16)
            for k in range(KC):
                nc.sync.dma_start(out=xr_sb[:, k, :], in_=xr_d[k * 128:(k + 1) * 128, :])
                nc.sync.dma_start(out=adjT_sb[:, k, :], in_=adjT_d[k * 128:(k + 1) * 128, :])

            xt_bf_sb = big.tile([C, RCOLS], BF16)
            nc.sync.dma_start(out=xt_bf_sb[:], in_=xt_bf_d[:])
            xtbo_sb = big.tile([C, RCOLS], F32)
            nc.sync.dma_start(out=xtbo_sb[:], in_=xtbo_d[:])
            degb0a_sb = big.tile([C, NS], F32)
            nc.sync.dma_start(out=degb0a_sb[:], in_=degb0a_d[:])
            wx_sb = const.tile([C, C], BF16)
            nc.sync.dma_start(out=wx_sb[:], in_=wx_d[:])
            wa_sb = const.tile([C, C], BF16)
            nc.sync.dma_start(out=wa_sb[:], in_=wa_d[:])
            wo_sb = const.tile([C, C], BF16)
            nc.sync.dma_start(out=wo_sb[:], in_=wo_d[:])
            b12_sb = const.tile([C, 1], F32)
            nc.sync.dma_start(out=b12_sb[:], in_=b12_d[:])

            ident = const.tile([128, 128], BF16)
            make_identity(nc, ident[:])

            # ---- s = adj @ x_r   [NS, COLS] accumulated over KC ------
            s_sb = big.tile([128, IC, COLS], BF16)
            for ic in range(IC):
                ps = [
                    ps_s.tile([128, 512], F32, tag="s_acc", name=f"s_acc_{ic}_{h}")
                    for h in range(2)
                ]
                for k in range(KC):
                    lhsT = adjT_sb[:, k, ic * 128:(ic + 1) * 128]
                    for h in range(2):
                        nc.tensor.matmul(
                            ps[h],
                            lhsT,
                            xr_sb[:, k, h * 512:(h + 1) * 512],
                            start=(k == 0),
                            stop=(k == KC - 1),
                        )
                for h in range(2):
                    dst = s_sb[:, ic, h * 512:(h + 1) * 512]
                    if h == 0:
                        nc.scalar.copy(out=dst, in_=ps[h])
                    else:
                        nc.vector.tensor_copy(out=dst, in_=ps[h])

            # ---- transpose s into sT [C, (b, row)] -------------------
            sT_sb = big.tile([C, RCOLS], BF16)
            for b in range(B):
                for ic in range(IC):
                    pt = ps_t.tile([128, 128], BF16, tag="t_ps")
                    nc.tensor.transpose(
                        pt[:], s_sb[:, ic, b * 128:(b + 1) * 128], ident[:]
                    )
                    dst = sT_sb[:, b * NS + ic * 128: b * NS + (ic + 1) * 128]
                    if (b * IC + ic) % 2 == 0:
                        nc.vector.tensor_copy(out=dst, in_=pt[:])
                    else:
                        nc.scalar.copy(out=dst, in_=pt[:])

            # ---- fused MLP: gelu(Wx.T xT + Wa.T sT + degb0a + b12) Wo
            gelu_sb = big.tile([C, RCOLS], BF16)
            res_sb = big.tile([C, RCOLS], F32)
            for b in range(B):
                cols = slice(b * NS, (b + 1) * NS)
                pp = ps_pre.tile([128, NS], F32, tag="pre")
                nc.tensor.matmul(pp, wx_sb[:], xt_bf_sb[:, cols], start=True, stop=False)
                nc.tensor.matmul(pp, wa_sb[:], sT_sb[:, cols], start=False, stop=True)
                nc.vector.tensor_add(out=pp[:], in0=pp[:], in1=degb0a_sb[:])
                nc.scalar.activation(
                    out=gelu_sb[:, cols], in_=pp[:],
                    func=mybir.ActivationFunctionType.Gelu,
                    bias=b12_sb[:, 0:1], scale=1.0,
                )
                po = ps_out.tile([128, NS], F32, tag="out")
                nc.tensor.matmul(po, wo_sb[:], gelu_sb[:, cols], start=True, stop=True)
                nc.vector.tensor_add(out=res_sb[:, cols], in0=po[:], in1=xtbo_sb[:, cols])
                nc.sync.dma_start(out=out_d[:, cols], in_=res_sb[:, cols])

    _split_multiwaits(nc)
    return nc


def host_prep(x, edge_index, W0, b0, W1, b1, W2, b2, Wo, bo):
    """Fold weights, build the dense adjacency, lay out per-core inputs."""
    x = np.asarray(x, np.float32)
    ei = np.asarray(edge_index, np.int64)
    W0, b0, W1, b1, W2, b2, Wo, bo = (
        np.asarray(a, np.float32) for a in (W0, b0, W1, b1, W2, b2, Wo, bo)
    )

    # dense symmetric adjacency with set-semantics dedup, zero diagonal
    k1 = ei[0] * N + ei[1]
    k2 = ei[1] * N + ei[0]
    keys = np.unique(np.concatenate([k1, k2]))
    rows = keys // N
    cols = keys % N
    off_diag = rows != cols
    keys, rows = keys[off_diag], rows[off_diag]
    adj = np.zeros(N * N, np.uint16)
    adj[keys] = 0x3F80  # bf16 1.0 bit pattern
    adj = adj.reshape(N, N).view(BF16_NP)
    deg = np.bincount(rows, minlength=N).astype(np.float32)

    # folded weights
    W12 = W1 @ W2                      # [2C, C]
    Wx = W12[:C]
    W12a = W12[C:]
    Wa = W0 @ W12a
    b0a = b0 @ W12a                    # [C]
    b12 = (b1 @ W2 + b2).reshape(C, 1)

    xr = np.ascontiguousarray(
        x.transpose(1, 0, 2).reshape(N, B * C)).astype(BF16_NP)   # [N,(b,c)]
    xt = x.transpose(2, 0, 1)                                     # [C,B,N] f32

    in_maps = []
    for c in range(NCORES):
        rs = slice(c * NS, (c + 1) * NS)
        xt_c = np.ascontiguousarray(xt[:, :, rs]).reshape(C, RCOLS)
        in_maps.append({
            "xr": xr,
            "adjT": np.ascontiguousarray(adj[:, rs]),
            "xt_bf": xt_c.astype(BF16_NP),
            "xtbo": np.ascontiguousarray(xt_c + bo[:, None]),
            "degb0a": np.ascontiguousarray(b0a[:, None] * deg[None, rs]),
            "wx": Wx.astype(BF16_NP),
            "wa": Wa.astype(BF16_NP),
            "wo": Wo.astype(BF16_NP),
            "b12": b12,
        })
    return in_maps


def assemble_output(results):
    out = np.empty((B, N, C), np.float32)
    for c in range(NCORES):
        r = results[c]["out"]                      # [C, (b, row)] f32
        out[:, c * NS:(c + 1) * NS, :] = r.reshape(C, B, NS).transpose(1, 2, 0)
    return out


_NC_CACHE = []


def kernel(x, edge_index, W0, b0, W1, b1, W2, b2, Wo, bo):
    in_maps = host_prep(x, edge_index, W0, b0, W1, b1, W2, b2, Wo, bo)
    if not _NC_CACHE:
        _NC_CACHE.append(build_bass())
    nc = _NC_CACHE[0]
    res = run_bass_kernel_spmd(nc, in_maps, list(range(NCORES)))
    return assemble_output(res.results)
